# revision 1
# baseline (speedup 1.0000x reference)
import sys

sys.path.insert(0, "/opt/trn_rl_repo")

import numpy as np
import ml_dtypes

from concourse import bass, mybir
from concourse.tile import TileContext
from concourse.bass_utils import run_bass_kernel_spmd

dt = mybir.dt
Alu = mybir.AluOpType
Act = mybir.ActivationFunctionType

H = 4096
W = 4096
NCORES = 8
RPC = H // NCORES            # 512 output rows per core
HALO = 4                     # blur(2) + sobel(1) + nms(1)
SH = RPC + 2 * HALO          # 520 input rows per core
BASES = (0, 120, 240, 360, 392)
NT = len(BASES)
NCH = 3
CW = 512
NCK = W // CW
P = 128
BF16 = ml_dtypes.bfloat16

TAN_LO2 = float(np.float32(np.tan(3.14159 / 8)) ** 2)
TAN_HI2 = float(np.float32(np.tan(3 * 3.14159 / 8)) ** 2)
LOWER_T = 6.0
UPPER_T = 50.0

# wb column layout: 5 blur bands then [V121, NV121, U, U2, SU, SD] x {mid, t0, t4}
GO_MID = 5 * P
GO_T0 = GO_MID + 6 * P
GO_T4 = GO_T0 + 6 * P
WBW = GO_T4 + 6 * P          # 2944


def _band(taps, r):
    # lhsT[k, m] = taps[k - m + r]  => out[m] = sum_k taps[k-m+r] * x[k]
    L = np.zeros((P, P), np.float32)
    for i, tv in enumerate(taps):
        L += np.float32(tv) * np.eye(P, k=r - i, dtype=np.float32)
    return L


def _weights(gauss, is_top, is_bot):
    g = np.asarray(gauss, np.float32)
    bg = _band(g, 2)
    v121 = _band([1.0, 2.0, 1.0], 1)
    u = _band([1.0, 0.0, -1.0], 1)
    su = _band([1.0], 1)
    sd = _band([1.0], -1)

    def group(zero_row, zero_su, zero_sd):
        mats = [v121.copy(), -v121, u.copy(), 2.0 * u, su.copy(), sd.copy()]
        if zero_row is not None:
            for idx in (0, 1, 2, 3):
                mats[idx][zero_row, :] = 0.0
            if zero_su:
                mats[4][zero_row, :] = 0.0
            if zero_sd:
                mats[5][zero_row, :] = 0.0
        return mats

    cols = [bg * g[d] for d in range(5)]
    cols += group(None, False, False)                       # mid
    cols += group(3 if is_top else None, True, False)       # t0 variant
    cols += group(124 if is_bot else None, False, True)     # t4 variant
    wb = np.concatenate(cols, axis=1)
    assert wb.shape == (P, WBW)
    return wb.astype(BF16)


def _build_nc():
    nc = bass.Bass(trn_type="TRN2")
    x_d = nc.dram_tensor("x", (NCH, SH, W + 4), dt.bfloat16, kind="ExternalInput")
    wb_d = nc.dram_tensor("wb", (P, WBW), dt.bfloat16, kind="ExternalInput")
    out_d = nc.dram_tensor("out", (NT * P, W), dt.uint8, kind="ExternalOutput")

    with TileContext(nc) as tc:
        with tc.tile_pool(name="sb", bufs=2) as pool, \
             tc.tile_pool(name="ps", bufs=2, space="PSUM") as pp:
            wb_sb = pool.tile([P, WBW], dt.bfloat16, tag="wb", bufs=1)
            nc.sync.dma_start(wb_sb[:, :], wb_d[:, :])

            for t in range(NT):
                base = BASES[t]
                go = GO_T0 if t == 0 else (GO_T4 if t == NT - 1 else GO_MID)

                # ---- stage A: fused separable 5x5 blur per channel ----
                blurs = []
                for c in range(NCH):
                    x_sb = pool.tile([P, W + 4], dt.bfloat16, tag="x", bufs=2)
                    nc.sync.dma_start(x_sb[:, :], x_d[c, base:base + P, :])
                    bl = pool.tile([P, W + 2], dt.bfloat16, tag="blur", bufs=4)
                    nc.vector.memset(bl[:, 0:1], 0.0)
                    nc.vector.memset(bl[:, W + 1:W + 2], 0.0)
                    for j0 in range(0, W, CW):
                        ps = pp.tile([P, CW], dt.float32, tag="psb", bufs=2)
                        for d in range(5):
                            nc.tensor.matmul(
                                out=ps[:, :],
                                lhsT=wb_sb[:, d * P:(d + 1) * P],
                                rhs=x_sb[:, j0 + d:j0 + d + CW],
                                start=(d == 0), stop=(d == 4),
                            )
                        nc.scalar.activation(bl[:, 1 + j0:1 + j0 + CW], ps[:, :], Act.Copy)
                    blurs.append(bl)

                # ---- stage B: grad, quadrant masks, vertical shifts ----
                blS = pool.tile([P, W + 2], dt.bfloat16, tag="blS", bufs=2)
                grad = pool.tile([P, W + 2], dt.bfloat16, tag="grad", bufs=2)
                gU = pool.tile([P, W + 2], dt.bfloat16, tag="gU", bufs=2)
                gD = pool.tile([P, W + 2], dt.bfloat16, tag="gD", bufs=2)
                for bufv in (blS, grad, gU, gD):
                    nc.vector.memset(bufv[:, 0:1], 0.0)
                    nc.vector.memset(bufv[:, W + 1:W + 2], 0.0)
                nc.vector.tensor_tensor(blS[:, 1:W + 1], blurs[0][:, 1:W + 1],
                                        blurs[1][:, 1:W + 1], Alu.add)
                nc.vector.tensor_tensor(blS[:, 1:W + 1], blS[:, 1:W + 1],
                                        blurs[2][:, 1:W + 1], Alu.add)
                csM = pool.tile([P, W], dt.uint8, tag="csM", bufs=2)
                c0M = pool.tile([P, W], dt.uint8, tag="c0M", bufs=2)
                c2M = pool.tile([P, W], dt.uint8, tag="c2M", bufs=2)

                for j0 in range(0, W, CW):
                    pj = 1 + j0
                    for c in range(NCH):
                        bl = blurs[c]
                        pgx = pp.tile([P, CW], dt.float32, tag="pgx", bufs=2)
                        nc.tensor.matmul(out=pgx[:, :], lhsT=wb_sb[:, go:go + P],
                                         rhs=bl[:, pj - 1:pj - 1 + CW], start=True, stop=False)
                        nc.tensor.matmul(out=pgx[:, :], lhsT=wb_sb[:, go + P:go + 2 * P],
                                         rhs=bl[:, pj + 1:pj + 1 + CW], start=False, stop=True)
                        pgy = pp.tile([P, CW], dt.float32, tag="pgy", bufs=2)
                        nc.tensor.matmul(out=pgy[:, :], lhsT=wb_sb[:, go + 2 * P:go + 3 * P],
                                         rhs=bl[:, pj - 1:pj - 1 + CW], start=True, stop=False)
                        nc.tensor.matmul(out=pgy[:, :], lhsT=wb_sb[:, go + 3 * P:go + 4 * P],
                                         rhs=bl[:, pj:pj + CW], start=False, stop=False)
                        nc.tensor.matmul(out=pgy[:, :], lhsT=wb_sb[:, go + 2 * P:go + 3 * P],
                                         rhs=bl[:, pj + 1:pj + 1 + CW], start=False, stop=True)
                        sqx = pool.tile([P, CW], dt.bfloat16, tag="sqx", bufs=2)
                        nc.scalar.activation(sqx[:, :], pgx[:, :], Act.Square)
                        sqy = pool.tile([P, CW], dt.bfloat16, tag="sqy", bufs=2)
                        nc.scalar.activation(sqy[:, :], pgy[:, :], Act.Square)
                        ss = pool.tile([P, CW], dt.bfloat16, tag="ss", bufs=2)
                        nc.vector.tensor_tensor(ss[:, :], sqx[:, :], sqy[:, :], Alu.add)
                        if c == 0:
                            nc.scalar.activation(grad[:, pj:pj + CW], ss[:, :], Act.Sqrt)
                        else:
                            mg = pool.tile([P, CW], dt.bfloat16, tag="mg", bufs=2)
                            nc.scalar.activation(mg[:, :], ss[:, :], Act.Sqrt)
                            nc.vector.tensor_tensor(grad[:, pj:pj + CW], grad[:, pj:pj + CW],
                                                    mg[:, :], Alu.add)
                    # gx/gy sums via conv linearity: one matmul set on summed blur
                    pgxs = pp.tile([P, CW], dt.float32, tag="pgx", bufs=2)
                    nc.tensor.matmul(out=pgxs[:, :], lhsT=wb_sb[:, go:go + P],
                                     rhs=blS[:, pj - 1:pj - 1 + CW], start=True, stop=False)
                    nc.tensor.matmul(out=pgxs[:, :], lhsT=wb_sb[:, go + P:go + 2 * P],
                                     rhs=blS[:, pj + 1:pj + 1 + CW], start=False, stop=True)
                    pgys = pp.tile([P, CW], dt.float32, tag="pgy", bufs=2)
                    nc.tensor.matmul(out=pgys[:, :], lhsT=wb_sb[:, go + 2 * P:go + 3 * P],
                                     rhs=blS[:, pj - 1:pj - 1 + CW], start=True, stop=False)
                    nc.tensor.matmul(out=pgys[:, :], lhsT=wb_sb[:, go + 3 * P:go + 4 * P],
                                     rhs=blS[:, pj:pj + CW], start=False, stop=False)
                    nc.tensor.matmul(out=pgys[:, :], lhsT=wb_sb[:, go + 2 * P:go + 3 * P],
                                     rhs=blS[:, pj + 1:pj + 1 + CW], start=False, stop=True)
                    gxsb = pool.tile([P, CW], dt.bfloat16, tag="gxsb", bufs=2)
                    nc.scalar.activation(gxsb[:, :], pgxs[:, :], Act.Copy)
                    gysb = pool.tile([P, CW], dt.bfloat16, tag="gysb", bufs=2)
                    nc.scalar.activation(gysb[:, :], pgys[:, :], Act.Copy)

                    sxy = pool.tile([P, CW], dt.bfloat16, tag="sxy", bufs=2)
                    nc.vector.tensor_tensor(sxy[:, :], gxsb[:, :], gysb[:, :], Alu.mult)
                    nc.vector.tensor_scalar(csM[:, j0:j0 + CW], sxy[:, :], 0.0, None, Alu.is_gt)
                    gx2 = pool.tile([P, CW], dt.bfloat16, tag="gx2", bufs=2)
                    nc.vector.tensor_tensor(gx2[:, :], gxsb[:, :], gxsb[:, :], Alu.mult)
                    gy2 = pool.tile([P, CW], dt.bfloat16, tag="gy2", bufs=2)
                    nc.vector.tensor_tensor(gy2[:, :], gysb[:, :], gysb[:, :], Alu.mult)
                    # c2: |gys| > |gxs|*T  <=>  gx2*T^2 < gy2
                    nc.vector.scalar_tensor_tensor(c2M[:, j0:j0 + CW], gx2[:, :], TAN_HI2,
                                                   gy2[:, :], Alu.mult, Alu.is_lt)
                    # c0: |gys| < |gxs|*t  <=>  gx2*t^2 > gy2
                    nc.vector.scalar_tensor_tensor(c0M[:, j0:j0 + CW], gx2[:, :], TAN_LO2,
                                                   gy2[:, :], Alu.mult, Alu.is_gt)

                    pU = pp.tile([P, CW], dt.float32, tag="pgx", bufs=2)
                    nc.tensor.matmul(out=pU[:, :], lhsT=wb_sb[:, go + 4 * P:go + 5 * P],
                                     rhs=grad[:, pj:pj + CW], start=True, stop=True)
                    nc.vector.tensor_scalar(gU[:, pj:pj + CW], pU[:, :], 1.0, None,
                                            Alu.mult)
                    pD = pp.tile([P, CW], dt.float32, tag="pgy", bufs=2)
                    nc.tensor.matmul(out=pD[:, :], lhsT=wb_sb[:, go + 5 * P:go + 6 * P],
                                     rhs=grad[:, pj:pj + CW], start=True, stop=True)
                    nc.vector.tensor_scalar(gD[:, pj:pj + CW], pD[:, :], 1.0, None,
                                            Alu.mult)

                # ---- stage C: NMS select + hysteresis band ----
                for j0 in range(0, W, CW):
                    pj = 1 + j0
                    m1 = pool.tile([P, CW], dt.bfloat16, tag="m1", bufs=2)
                    nc.vector.tensor_tensor(m1[:, :], gD[:, pj + 1:pj + 1 + CW],
                                            gU[:, pj - 1:pj - 1 + CW], Alu.max)
                    msel = pool.tile([P, CW], dt.bfloat16, tag="msel", bufs=2)
                    nc.vector.tensor_tensor(msel[:, :], gD[:, pj - 1:pj - 1 + CW],
                                            gU[:, pj + 1:pj + 1 + CW], Alu.max)
                    m0 = pool.tile([P, CW], dt.bfloat16, tag="m0", bufs=2)
                    nc.vector.tensor_tensor(m0[:, :], grad[:, pj - 1:pj - 1 + CW],
                                            grad[:, pj + 1:pj + 1 + CW], Alu.max)
                    m2u = pool.tile([P, CW], dt.bfloat16, tag="m2u", bufs=2)
                    nc.vector.tensor_tensor(m2u[:, :], gU[:, pj:pj + CW],
                                            gD[:, pj:pj + CW], Alu.max)
                    nc.vector.copy_predicated(msel[:, :], csM[:, j0:j0 + CW], m1[:, :])
                    nc.vector.copy_predicated(msel[:, :], c0M[:, j0:j0 + CW], m0[:, :])
                    nc.vector.copy_predicated(msel[:, :], c2M[:, j0:j0 + CW], m2u[:, :])
                    ig = pool.tile([P, CW], dt.bfloat16, tag="ig", bufs=2)
                    # is_max & grad > 6:  max(msel, 6) < grad
                    nc.vector.scalar_tensor_tensor(ig[:, :], msel[:, :], LOWER_T,
                                                   grad[:, pj:pj + CW], Alu.max, Alu.is_lt)
                    ob = pool.tile([P, CW], dt.uint8, tag="ob", bufs=2)
                    # (grad <= 50) * ig
                    nc.vector.scalar_tensor_tensor(ob[:, :], grad[:, pj:pj + CW], UPPER_T,
                                                   ig[:, :], Alu.is_le, Alu.mult)
                    nc.sync.dma_start(out_d[t * P:(t + 1) * P, j0:j0 + CW], ob[:, :])

    import bass_rust
    # HW descriptors hold only one sync wait; park extras on Ldweights /
    # split the remainder into EventSemaphore instructions
    bass_rust.move_matmul_waits_to_ldweights(nc.m)
    bass_rust.generate_event_semaphores(nc)
    nc.finalize()
    return nc


def _shard_inputs(img, gauss):
    imgf = np.ascontiguousarray(img[0])  # [3, H, W] f32
    in_maps = []
    for k in range(NCORES):
        xk = np.zeros((NCH, SH, W + 4), dtype=BF16)
        lo = k * RPC - HALO
        hi = k * RPC + RPC + HALO
        s0, s1 = max(lo, 0), min(hi, H)
        xk[:, s0 - lo:s1 - lo, 2:W + 2] = imgf[:, s0:s1, :].astype(BF16)
        wbk = _weights(gauss, is_top=(k == 0), is_bot=(k == NCORES - 1))
        in_maps.append({"x": xk, "wb": wbk})
    return in_maps


def _assemble(results):
    full = np.zeros((H, W), dtype=np.float32)
    for k in range(NCORES):
        ok = np.asarray(results[k]["out"])
        r0 = k * RPC
        for t in range(NT - 1):
            full[r0 + 120 * t:r0 + 120 * t + 120] = ok[P * t + 4:P * t + 124]
        full[r0 + 480:r0 + 512] = ok[(NT - 1) * P + 92:(NT - 1) * P + 124]
    return full.reshape(1, 1, H, W)


def _run(img, gauss, trace=False):
    nc = _build_nc()
    in_maps = _shard_inputs(np.asarray(img, np.float32), np.asarray(gauss, np.float32))
    res = run_bass_kernel_spmd(nc, in_maps, core_ids=list(range(NCORES)), trace=trace)
    return _assemble(res.results), res.exec_time_ns


def kernel(img=None, gauss=None, sobel=None, dir_w=None, **_):
    out, _ns = _run(img, gauss)
    return out

